# revision 94
# baseline (speedup 1.0000x reference)
"""Causal multi-head attention on 8 trn2 NeuronCores.

Problem (hardcoded): x[4, 2048, 768], w_attn[768, 2304], b_attn[2304],
w_proj[768, 768], b_proj[768]; H=6 heads, D=128 head dim; fp32 in/out.

Sharding: core c = 2*b + g handles batch b and head-group g (heads
3g..3g+2).  Each core computes Q/K/V projections for its 3 heads over the
full sequence, full causal attention for those heads, and a PARTIAL output
projection (w_proj rows of its heads).  The host sums the two partials per
batch and adds the bias terms.  No cross-core communication.

Bias algebra (host/device split):
  - b_q is added on device (affects scores).
  - b_k is dropped entirely: it shifts every score in a softmax row by the
    same constant, which cancels.
  - b_v is dropped on device: softmax rows sum to 1, so attn @ (v + b_v)
    = attn @ v + b_v; the constant (b_v @ w_proj + b_proj) is added on host.

Precision plan (HW rel err ~3e-3 vs the 2e-2 gate):
  - QKV projection runs as fp8e4m3 DoubleRow matmuls (2 contraction chunks
    per pass, 0.5 cycles/row = 4x fp32r throughput) using a hi+lo residual
    split of BOTH x and w_attn: x ~= xh + xl, w ~= wh + wl (each fp8), and
    q = xh@wh + xh@wl + xl@wh (the lo*lo term is ~1e-5 and dropped).  Nine
    DR matmuls replace six bf16 ones: 25% fewer PE cycles, same DMA bytes
    as bf16, all splits prepared on the host.  w_attn is pre-scaled by 32
    so its residual stays out of the fp8 subnormal floor; the 32x on q/k
    is folded into the exp scale and the 32x on v into w_proj (pre-divided
    by 32 on the host).
  - Everything else is bf16 operands with fp32 PSUM accumulation: scores,
    PV, output projection (1 cycle/row, same as fp32r, but half the SBUF
    and DMA).  fp8 anywhere else fails the error gate (outlier softmax
    rows: max|out|/rms ~ 32).

Softmax denominators: instead of 40 ones-matmuls per head (as many PE
cycles as PV itself), the exp'd score chunks are accumulated on the DVE
into a per-(group,head) fp16 tile (2-byte dtype = 2x DVE mode), and ONE
ones-matmul per (group,head) does the final 128-partition reduction,
replicating the denominators across partitions for the divide.

Causal structure: group t attends kv [0, 512(t+1)); diagonal kv chunk k
(k = j-(nk-4)) only attends query columns [128k, 512), so scores/exp/PV/
denominator-accumulation all run on that suffix (-15% attention work),
and the triangle inside its first 128 valid columns is fixed by ONE
shared [128,128] 0/1 bf16 mask multiplied into ex (DVE 2x mode).

Engine budget per core (cost model): PE ~96us busy (QKV 35 + scores 22 +
PV 22 + rowsum 2.6 + proj 15), DVE ~79us (masks, denominators, divides,
filler copies), Act ~74us (exp + inline qkv-psum copies), total 109.9us.
Startup DMA triggers are spread across the SP, Act, and Pool sequencers
(all idle at t=0) so the input loads issue in parallel instead of
serializing ~1.1us each on SP.

Scheduling: inputs split into DMAs ordered by first use; the attention
inner loop keeps a FOUR-batch software-pipeline skew (PE is in-order) so
exp latency never stalls PE.  QKV groups 2-3 (attention group t only
reads QKV groups <= t) and the output-projection blocks are deferred
into a filler queue popped late in each attention head, exactly where
the slower Act engine (exp: 1028ns vs PE 853ns per kv-chunk pair) falls
behind; this also starts the exp pipeline two QKV groups earlier.
Deferred tiles and proj blocks allocate PSUM from the short-lived "rs"
ring (every occupant's reader is emitted immediately after its writer) —
putting them in the "pv" ring deadlocks against the deferred finalize
whose divide frees the slot.
"""

import math
from contextlib import ExitStack

import numpy as np
import ml_dtypes

import concourse.bacc as bacc
import concourse.bass as bass
import concourse.mybir as mybir
import concourse.tile as tile
from concourse import bass_utils

B, S, C = 4, 2048, 768
H, D = 6, 128
HL = 3          # heads per core
CK = C // 128   # 6 contraction chunks
CP = CK // 2    # 3 chunk pairs for DoubleRow
R = 512         # query rows per group
G = S // R      # 4 groups
N_CORES = 8
F32 = mybir.dt.float32
BF16 = mybir.dt.bfloat16
FP16 = mybir.dt.float16
FP8 = mybir.dt.float8e4
NP_FP8 = ml_dtypes.float8_e4m3
NP_BF16 = ml_dtypes.bfloat16
WS = 32.0                      # w_attn pre-scale (fp8 subnormal avoidance)
INV_SQRT_D = 1.0 / math.sqrt(D)
EXP_SCALE = INV_SQRT_D / (WS * WS)
MASKW = 128                    # one shared [128,128] causal triangle mask
DR = mybir.MatmulPerfMode.DoubleRow


def _emit(ctx: ExitStack, tc: tile.TileContext, xa, wav, wqk,
          bq, mask01, ones, wp, outT):
    nc = tc.nc

    singles = ctx.enter_context(tc.tile_pool(name="singles", bufs=1))
    expool = ctx.enter_context(tc.tile_pool(name="expool", bufs=8))
    aopool = ctx.enter_context(tc.tile_pool(name="aopool", bufs=2))
    otpool = ctx.enter_context(tc.tile_pool(name="otpool", bufs=4))
    rspool = ctx.enter_context(tc.tile_pool(name="rspool", bufs=3))
    accpool = ctx.enter_context(tc.tile_pool(name="accpool", bufs=3))
    psum = ctx.enter_context(tc.tile_pool(name="psum", bufs=2, space="PSUM"))

    # ---- resident loads, split + ordered by first use ----
    xa_sb = singles.tile([128, G, 2, CK, R], FP8)
    wav_sb = singles.tile([128, 2, CK, HL * D], FP8)
    wqk_sb = singles.tile([128, 2, CK, 2 * HL * D], FP8)
    bq_sb = singles.tile([128, HL], F32)
    mask_sb = singles.tile([128, MASKW], BF16)
    ones_sb = singles.tile([128, 128], FP16)
    wp_sb = singles.tile([128, HL, C], BF16)

    def load_half(n, hi, eng=None):
        w, half = 2 * CK * R, CK * R
        (eng or nc.sync).dma_start(
            xa_sb[:, n, hi],
            xa[:, n * w + hi * half:n * w + (hi + 1) * half]
            .rearrange("p (c s) -> p c s", c=CK))

    def load_whalf(w_sb, w_dram, hi, width, eng=None):
        (eng or nc.sync).dma_start(
            w_sb[:, hi],
            w_dram[:, hi * CK * width:(hi + 1) * CK * width]
            .rearrange("p (c n) -> p c n", c=CK))

    # Startup loads issued from FOUR different sequencers (SP, Act, DVE,
    # Pool are all idle at t=0), so the DMA triggers go out in parallel
    # instead of serializing ~1.1us each on SP.  All group-0 inputs land
    # within ~2us.
    load_half(0, 0)                                    # SP
    load_whalf(wav_sb, wav, 0, HL * D, nc.scalar)      # Act
    load_whalf(wav_sb, wav, 1, HL * D, nc.scalar)
    load_half(0, 1, nc.gpsimd)                         # Pool
    load_whalf(wqk_sb, wqk, 0, 2 * HL * D, nc.scalar)
    load_whalf(wqk_sb, wqk, 1, 2 * HL * D, nc.gpsimd)
    nc.gpsimd.dma_start(bq_sb, bq)
    nc.gpsimd.dma_start(mask_sb, mask01)
    nc.gpsimd.dma_start(ones_sb, ones)
    for n in range(1, G):
        load_half(n, 0)
        load_half(n, 1, nc.gpsimd)
    nc.gpsimd.dma_start(wp_sb, wp.rearrange("p (f n) -> p f n", f=HL))

    # ---- QKV projections: 9 fp8 DoubleRow matmuls per output tile
    # (3 chunk pairs x 3 hi/lo cross terms), interleaved per x token-chunk
    # so PE work tracks DMA arrival.
    V_sb = singles.tile([128, S // 128, HL * D], BF16)
    qkT_sb = singles.tile([128, 2 * HL, S], BF16)

    def v_terms(ps, n, r, terms, start, stop):
        idx = 0
        nt = len(terms) * CP
        for xi, wi in terms:
            for cp in range(CP):
                nc.tensor.matmul(
                    ps[:, :HL * D],
                    lhsT=xa_sb[:, n, xi, 2 * cp:2 * cp + 2,
                               (r % 4) * 128:(r % 4 + 1) * 128],
                    rhs=wav_sb[:, wi, 2 * cp:2 * cp + 2, :],
                    start=(start and idx == 0),
                    stop=(stop and idx == nt - 1),
                    perf_mode=DR,
                )
                idx += 1

    def v_tile(n, r, tag):
        ps = psum.tile([128, R], F32, tag=tag)
        v_terms(ps, n, r, ((0, 0), (0, 1), (1, 0)), True, True)
        if tag == "rs":  # filler tile mid-attention: keep Act (exp) free
            nc.vector.tensor_copy(V_sb[:, r, :], ps[:, :HL * D])
        else:
            nc.scalar.copy(V_sb[:, r, :], ps[:, :HL * D])



    def qk_tile(n, f, tag):
        ps = psum.tile([128, R], F32, tag=tag)
        idx = 0
        for wi, xi in ((0, 0), (0, 1), (1, 0)):
            for cp in range(CP):
                nc.tensor.matmul(
                    ps,
                    lhsT=wqk_sb[:, wi, 2 * cp:2 * cp + 2, f * 128:(f + 1) * 128],
                    rhs=xa_sb[:, n, xi, 2 * cp:2 * cp + 2, :],
                    start=(idx == 0),
                    stop=(idx == 3 * CP - 1),
                    perf_mode=DR,
                )
                idx += 1
        dst = qkT_sb[:, f, n * R:(n + 1) * R]
        if f >= HL:  # k: plain copy
            if tag == "rs":  # filler tile mid-attention: keep Act (exp) free
                nc.vector.tensor_copy(dst, ps)
            else:
                nc.scalar.copy(dst, ps)
        elif tag == "rs":
            nc.vector.tensor_scalar_add(dst, ps, bq_sb[:, f:f + 1])
        else:
            nc.scalar.add(dst, ps, bq_sb[:, f:f + 1])

    # groups 0-1 inline; groups 2 and 3 become filler work popped inside
    # the attention pipeline (attention group t only reads QKV groups <= t,
    # and the per-head Act-lag stalls are exactly PE-sized holes).  This
    # also starts the exp pipeline two QKV groups earlier.
    for n in range(2):
        for r in range(4 * n, 4 * n + 4):
            v_tile(n, r, "st")
        for f in range(2 * HL):
            qk_tile(n, f, "st")
    qkv_fill = []
    for n in (2, 3):
        qkv_fill += [(n, lambda n=n, r=r: v_tile(n, r, "rs"))
                     for r in range(4 * n, 4 * n + 4)]
        qkv_fill += [(n, lambda n=n, f=f: qk_tile(n, f, "rs"))
                     for f in range(2 * HL)]

    # ---- attention + output projection, software-pipelined ----
    pending = []
    proj_queue = []

    def push(fn):
        pending.append(fn)
        while len(pending) > 4:  # 4-deep skew: PV/finalize trail the score
            pending.pop(0)()     # matmuls by 4 batches (ex ring is 8)

    def pop_fill(k):
        for _ in range(k):
            if qkv_fill:
                qkv_fill.pop(0)[1]()
            elif proj_queue:
                proj_queue.pop(0)()

    def drain():
        while pending:
            pending.pop(0)()
        while proj_queue:
            proj_queue.pop(0)()

    # Diagonal kv chunk k (k = j-(nk-4)) only attends query columns
    # [128k, 512): scores/exp/PV/acc all run on that suffix, and the causal
    # triangle inside its first 128 columns is fixed by ONE shared [128,128]
    # 0/1 mask.  Chunk j=0 is always full-width, so the PV psum start=True
    # initializes the whole [128, R] region.
    for t in (0, 1, 2, 3):
        while qkv_fill and qkv_fill[0][0] <= t:
            qkv_fill.pop(0)[1]()  # attention t reads QKV groups <= t
        rows = t * R
        nk = 4 * (t + 1)

        def off(j, nk=nk):
            return 128 * (j - (nk - 4)) if j >= nk - 4 else 0

        ao = aopool.tile([128, HL, R], BF16, tag="ao")
        for h in range(HL):
            pv = psum.tile([128, R], F32, tag="pv")
            acc = accpool.tile([128, R], FP16, tag="acc")
            for jb in range(nk // 2):
                # late fillers land where the Act lag peaks; deferred
                # QKV-g3 tiles first, then queued proj blocks
                if jb >= nk // 2 - (3 if t == 3 else 2):
                    pop_fill(1)
                if jb == nk // 2 - 1 and t != 3:
                    pop_fill(2 if t == 2 else 1)
                o0, o1 = off(2 * jb), off(2 * jb + 1)
                st = psum.tile([128, 2, R], F32, tag="st")
                for u, o in ((0, o0), (1, o1)):
                    j = 2 * jb + u
                    nc.tensor.matmul(
                        st[:, u, o:],
                        lhsT=qkT_sb[:, HL + h, j * 128:(j + 1) * 128],
                        rhs=qkT_sb[:, h, rows + o:rows + R],
                        start=True,
                        stop=True,
                    )
                ex = expool.tile([128, 2, R], BF16, tag="ex")
                # one exp over the common suffix, plus chunk0's head columns
                nc.scalar.activation(
                    ex[:, :, o1:], st[:, :, o1:],
                    mybir.ActivationFunctionType.Exp, scale=EXP_SCALE,
                )
                if o1 > o0:
                    nc.scalar.activation(
                        ex[:, 0, o0:o1], st[:, 0, o0:o1],
                        mybir.ActivationFunctionType.Exp, scale=EXP_SCALE,
                    )
                # causal fix-up: 0/1 triangle mult on each diagonal chunk's
                # first 128 valid columns (bf16 all-SBUF = DVE 2x mode)
                for u, o in ((0, o0), (1, o1)):
                    j = 2 * jb + u
                    if j >= nk - 4:
                        nc.vector.tensor_tensor(
                            ex[:, u, o:o + 128], ex[:, u, o:o + 128],
                            mask_sb, mybir.AluOpType.mult,
                        )
                # fp16 denominator accumulation on DVE (2x mode): tmp is
                # independent per jb; only the acc += tmp adds are chained
                if jb == 0:
                    if o1 == 0:
                        nc.vector.tensor_tensor(
                            acc, ex[:, 0, :], ex[:, 1, :], mybir.AluOpType.add)
                    else:  # t == 0: ragged first pair
                        nc.vector.tensor_copy(acc, ex[:, 0, :])
                        nc.vector.tensor_tensor(
                            acc[:, o1:], acc[:, o1:], ex[:, 1, o1:],
                            mybir.AluOpType.add)
                elif o0 == o1 == 0:
                    tmp = accpool.tile([128, R], FP16, tag="tmp")
                    nc.vector.tensor_tensor(
                        tmp, ex[:, 0, :], ex[:, 1, :], mybir.AluOpType.add)
                    nc.vector.tensor_tensor(acc, acc, tmp, mybir.AluOpType.add)
                else:  # diagonal pair: two direct suffix adds
                    nc.vector.tensor_tensor(
                        acc[:, o0:], acc[:, o0:], ex[:, 0, o0:],
                        mybir.AluOpType.add)
                    nc.vector.tensor_tensor(
                        acc[:, o1:], acc[:, o1:], ex[:, 1, o1:],
                        mybir.AluOpType.add)

                def consume(jb=jb, h=h, pv=pv, ex=ex, nk=nk, o0=o0, o1=o1):
                    for u, o in ((0, o0), (1, o1)):
                        j = 2 * jb + u
                        nc.tensor.matmul(
                            pv[:, o:],
                            lhsT=V_sb[:, j, h * D:(h + 1) * D],
                            rhs=ex[:, u, o:],
                            start=(j == 0),
                            stop=(j == nk - 1),
                        )

                push(consume)

            def finalize(h=h, t=t, pv=pv, acc=acc, ao=ao):
                rs = psum.tile([128, R], F32, tag="rs")
                nc.tensor.matmul(rs, lhsT=ones_sb, rhs=acc, start=True, stop=True)
                rsr = rspool.tile([128, R], F32, tag="rsr")
                nc.vector.reciprocal(rsr, rs)
                nc.vector.tensor_tensor(
                    ao[:, h, :], pv, rsr, mybir.AluOpType.mult)
                if h == HL - 1:
                    proj_queue.extend(
                        _proj_obs(nc, psum, otpool, wp_sb, ao, outT, t))

            push(finalize)
    drain()


def _proj_obs(nc, psum, otpool, wp_sb, ao, outT, t):
    rows = slice(t * R, (t + 1) * R)
    last = t == G - 1

    def one(ob, cl=0, ch=R):
        w = ch - cl
        ps = psum.tile([128, R], mybir.dt.float32, tag="rs")
        for fc in range(HL):
            nc.tensor.matmul(
                ps[:, :w],
                lhsT=wp_sb[:, fc, ob * 128:(ob + 1) * 128],
                rhs=ao[:, fc, cl:ch],
                start=(fc == 0),
                stop=(fc == HL - 1),
            )
        ot = otpool.tile([128, R], BF16, tag="ot")
        # last group's copies drain after all other work: split across
        # Act (idle by then) and DVE so the tail halves
        if last and ob % 2 == 0:
            nc.scalar.copy(ot[:, :w], ps[:, :w])
        else:
            nc.vector.tensor_copy(ot[:, :w], ps[:, :w])
        nc.sync.dma_start(
            outT[ob * 128:(ob + 1) * 128, t * R + cl:t * R + ch], ot[:, :w])

    return [lambda ob=ob: one(ob) for ob in range(C // 128)]


_CACHED = None


def _build():
    global _CACHED
    if _CACHED is not None:
        return _CACHED
    nc = bacc.Bacc(
        "TRN2",
        target_bir_lowering=False,
        debug=False,
        enable_asserts=False,
        num_devices=N_CORES,
    )
    xa = nc.dram_tensor("xa", [128, G * 2 * CK * R], FP8, kind="ExternalInput").ap()
    wav = nc.dram_tensor("wav", [128, 2 * CK * HL * D], FP8, kind="ExternalInput").ap()
    wqk = nc.dram_tensor("wqk", [128, 2 * CK * 2 * HL * D], FP8, kind="ExternalInput").ap()
    bq = nc.dram_tensor("bq", [128, HL], F32, kind="ExternalInput").ap()
    mask01 = nc.dram_tensor("mask01", [128, MASKW], BF16, kind="ExternalInput").ap()
    ones = nc.dram_tensor("ones", [128, 128], FP16, kind="ExternalInput").ap()
    wp = nc.dram_tensor("wp", [128, HL * C], BF16, kind="ExternalInput").ap()
    outT = nc.dram_tensor("outT", [C, S], BF16, kind="ExternalOutput").ap()
    with tile.TileContext(nc) as tc, ExitStack() as ctx:
        _emit(ctx, tc, xa, wav, wqk, bq, mask01, ones, wp, outT)
    nc.compile()
    _CACHED = nc
    return nc


def _pmajor(a2d):
    """[n*128, w] -> [128, n*w]: partition-major shuffle for one-DMA loads."""
    n = a2d.shape[0] // 128
    w = a2d.shape[1]
    return np.ascontiguousarray(
        a2d.reshape(n, 128, w).transpose(1, 0, 2).reshape(128, n * w))


def _hilo(a):
    """fp32 array -> (hi, lo) fp8e4m3 pair with hi + lo ~= a."""
    hi = a.astype(NP_FP8)
    lo = (a - hi.astype(np.float32)).astype(NP_FP8)
    return hi, lo


def _masks01():
    """Shared 0/1 bf16 causal triangle: mask[p, q] = 1 iff q >= p."""
    p = np.arange(128)[:, None]
    q = np.arange(128)[None, :]
    return (q >= p).astype(NP_BF16)  # [128, 128]


def shard_inputs(x, w_attn, b_attn, w_proj):
    """Per-core input dicts for cores 0..7 (core = 2*batch + head_group)."""
    masks = _masks01()
    ones = np.ones((128, 128), np.float16)
    xs = []
    for b in range(B):
        xT = np.ascontiguousarray(x[b].T)  # [768, 2048]
        xh, xl = _hilo(xT)
        # per group: [hi block | lo block], each chunk-major
        xs.append(np.concatenate(
            [_pmajor(xq[:, n * CK * 0 + n * R:(n + 1) * R])
             for n in range(G) for xq in (xh, xl)], axis=1))
    in_maps = []
    for c in range(N_CORES):
        b, g = divmod(c, 2)
        lo, hi = g * HL * D, (g + 1) * HL * D
        wav = w_attn[:, 2 * C + lo:2 * C + hi] * WS
        wqk = np.concatenate(
            [w_attn[:, lo:hi], w_attn[:, C + lo:C + hi]], axis=1) * WS
        wavh, wavl = _hilo(wav)
        wqkh, wqkl = _hilo(wqk)
        bqv = (WS * b_attn[lo:hi]).astype(np.float32).reshape(HL, 128).T
        in_maps.append({
            "xa": xs[b],
            "wav": np.concatenate([_pmajor(wavh), _pmajor(wavl)], axis=1),
            "wqk": np.concatenate([_pmajor(wqkh), _pmajor(wqkl)], axis=1),
            "bq": np.ascontiguousarray(bqv),
            "mask01": masks,
            "ones": ones,
            "wp": _pmajor((w_proj[lo:hi, :] / WS).astype(NP_BF16)),
        })
    return in_maps


def combine_outputs(parts, b_attn, w_proj, b_proj):
    """parts[c] = outT partial [768, 2048] (bf16) from core c."""
    bias = b_attn[2 * C:].astype(np.float64) @ w_proj.astype(np.float64) + b_proj
    out = np.empty((B, S, C), np.float32)
    for b in range(B):
        acc = parts[2 * b].astype(np.float32) + parts[2 * b + 1].astype(np.float32)
        out[b] = acc.T + bias.astype(np.float32)[None, :]
    return out


def kernel(x, w_attn, b_attn, w_proj, b_proj, **run_kwargs):
    x = np.asarray(x, np.float32)
    w_attn = np.asarray(w_attn, np.float32)
    b_attn = np.asarray(b_attn, np.float32)
    w_proj = np.asarray(w_proj, np.float32)
    b_proj = np.asarray(b_proj, np.float32)

    nc = _build()
    in_maps = shard_inputs(x, w_attn, b_attn, w_proj)
    res = bass_utils.run_bass_kernel_spmd(
        nc, in_maps, core_ids=list(range(N_CORES)), **run_kwargs
    )
    parts = [r["outT"] for r in res.results]
    out = combine_outputs(parts, b_attn, w_proj, b_proj)
    kernel.last_results = res
    return out
